# revision 47
# baseline (speedup 1.0000x reference)
"""Trainium2 Bass kernel for nn_BoxMpcController.

Solves a batched box-constrained MPC QP via 12 fixed primal-dual IPM
iterations, matching reference.py's trajectory exactly (to f32 noise).

Math (host precompute from the fixed module params A,B,C,Q,R):
  The per-iteration KKT system is an LQR two-point boundary problem whose
  only non-constant part is a per-stage 4x4 diagonal shift of the input
  Hessian.  Device algorithm per IPM iteration:
    residuals (PE matmuls with constant weights)
    backward Riccati over H=16 stages (lane-major DVE + PE sandwich)
    forward rollout, dual recovery, fraction-to-boundary step (DVE)
Sharding: pure data parallel, batch 128 -> 8 cores x 16 lanes.
"""
import sys

sys.path.insert(0, "/opt/trn_rl_repo")

import numpy as np

X = 8
U = 4
SD = 2
H = 16
NX = H * X          # 128
NU = H * U          # 64
M2 = 2 * NU         # 128
NITER = 12
SIGMA = 0.1
NLANES = 16
NCORES = 8
U_UB = 1.0
U_LB = -1.0


# ---------------------------------------------------------------- host consts
def build_host_consts(A, Bm, C, Q, R):
    A, Bm, C, Q, R = [np.asarray(v, np.float64) for v in (A, Bm, C, Q, R)]
    IH = np.eye(H)
    Q_aug = np.kron(IH, Q)
    R_aug = np.kron(IH, R)
    B_aug = np.kron(IH, Bm)
    C_aug = np.kron(IH, C)
    L = np.kron(IH, np.eye(X)) - np.kron(np.eye(H, k=-1), A)
    l_aug = np.zeros((NX, X)); l_aug[:X] = A
    Linv = np.linalg.inv(L)
    F = Linv @ B_aug

    f32 = lambda a: np.ascontiguousarray(a, np.float32)

    # lane-major producing matmuls (state as stationary, const as moving):
    consts = {
        "nQaug": f32(-Q_aug),            # rhs [128,128]
        "nL": f32(-L),                   # rhs [128,128]
        "LTr": f32(L.T),                 # rhs [128,128]
        "nR": f32(-R_aug),               # rhs [64,64]
        "Br": f32(B_aug),                # rhs [128,64]
        "FT": f32(F.T),                  # lhsT [64,128] -> out = F @ u
        "WinLb": f32(np.vstack([l_aug.T, C_aug.T]) @ Linv.T),  # [40,128]
    }

    # Riccati sandwich weights: [65, 112]
    # output col layout: [0:64] vec(Q+A^T P A);  [64:112] per-row-i (i in 4):
    #   [ (B^T P A)[i, 0:8] | (R + B^T P B)[i, 0:4] ]  (12 cols per i)
    def interleave_GU(Gblk, Ublk):
        # per-row concat: [..., 4, a], [..., 4, b] -> [..., 4*(a+b)]
        cat = np.concatenate([Gblk, Ublk], axis=-1)
        return cat.reshape(*cat.shape[:-2], cat.shape[-2] * cat.shape[-1])

    W1 = np.einsum("ai,bj->abij", A, A).reshape(64, 64)
    W2 = np.einsum("ai,bj->abij", Bm, A).reshape(64, 4, 8)
    W3 = np.einsum("ai,bj->abij", Bm, Bm).reshape(64, 4, 4)
    W23 = interleave_GU(np.concatenate(
        [W2, np.zeros((64, 4, 4))], axis=2), W3)       # [64, 64]
    crowGU = interleave_GU(np.concatenate(
        [np.zeros((1, 4, 8)), np.eye(4)[None]], axis=2),
        R.reshape(1, 4, 4))[0]
    crow = np.concatenate([Q.reshape(64), crowGU])
    consts["Wsand"] = f32(np.vstack([np.hstack([W1, W23]), crow]))

    rep = lambda a: f32(np.broadcast_to(a.reshape(1, -1), (NLANES, a.size)))
    # first backward step (P = Q) sandwich output, constant:
    AtQA = A.T @ Q @ A + Q
    GQ = Bm.T @ Q @ A
    BtQB = R + Bm.T @ Q @ Bm
    consts["SL0"] = rep(np.concatenate(
        [AtQA.reshape(64),
         interleave_GU(np.concatenate(
             [GQ.reshape(1, 4, 8), np.eye(4)[None]], axis=2),
             BtQB.reshape(1, 4, 4))[0]]))
    consts["vecQ"] = rep(Q.reshape(64))
    # (A^T; B^T) stacked, i-major (i in 12, k in 8)
    ABr = np.concatenate([A.T, Bm.T], axis=0)          # [12, 8]
    consts["ABrep"] = rep(ABr.reshape(-1))
    # [A B] stacked (i in 8, k in 12)
    FWr = np.concatenate([A, Bm], axis=1)              # [8, 12]
    consts["FWrep"] = rep(FWr.reshape(-1))
    consts["Brep2"] = rep(np.asarray(Bm).reshape(-1))  # [16, 32] i-major
    consts["Arep2"] = rep(np.asarray(A).reshape(-1))   # [16, 64] i-major
    return consts


# ---------------------------------------------------------------- bass program
CFG = {"mchain": "g", "Mchain": "g", "pchain": "g", "F": "g"}


def build_program(n_iters=NITER, dbg=False, cfg=None):
    import concourse.bacc as bacc
    import concourse.mybir as mybir
    from concourse.masks import make_identity
    from concourse.tile import TileContext

    f32 = mybir.dt.float32
    AL = mybir.AluOpType
    AX = mybir.AxisListType

    cfg = dict(CFG if cfg is None else cfg)
    nc = bacc.Bacc("TRN2", target_bir_lowering=False, debug=False,
                   num_devices=NCORES)

    def eng(key):
        return {"g": nc.gpsimd, "v": nc.vector}[cfg.get(key, "v")]

    # ---- dram I/O
    din = {}
    for name, shape in [("nQaug", [128, 128]), ("nL", [128, 128]),
                        ("LTr", [128, 128]), ("nR", [64, 64]),
                        ("Br", [128, 64]), ("FT", [64, 128]),
                        ("WinLb", [40, 128]), ("Wsand", [65, 128]),
                        ("SL0", [NLANES, 128]), ("vecQ", [NLANES, 64]),
                        ("ABrep", [NLANES, 96]), ("FWrep", [NLANES, 96]),
                        ("Brep2", [NLANES, 32]), ("Arep2", [NLANES, 64]),
                        ("xin", [40, NLANES])]:
        din[name] = nc.dram_tensor(name, shape, f32, kind="ExternalInput")
    x_out = nc.dram_tensor("x_out", [NLANES, NX], f32, kind="ExternalOutput")
    u_out = nc.dram_tensor("u_out", [NLANES, NU], f32, kind="ExternalOutput")

    from contextlib import ExitStack
    with TileContext(nc) as tc, ExitStack() as ctx:
        consts = ctx.enter_context(tc.tile_pool(name="consts", bufs=1))
        state = ctx.enter_context(tc.tile_pool(name="state", bufs=1))
        wk = ctx.enter_context(tc.tile_pool(name="work", bufs=2))
        ps = ctx.enter_context(tc.tile_pool(name="psum", bufs=1, space="PSUM"))

        # ---- const tiles
        ct = {}
        for name in ["nQaug", "nL", "LTr", "nR", "Br", "FT", "WinLb",
                     "Wsand", "SL0", "vecQ", "ABrep", "FWrep", "Brep2",
                     "Arep2", "xin"]:
            shape = list(din[name].shape)
            ct[name] = consts.tile(shape, f32, name=f"ct_{name}")
            nc.sync.dma_start(ct[name][:], din[name][:])
        ident = consts.tile([128, 128], f32)
        make_identity(nc, ident)

        # ---- state tiles (persistent)
        x_lm = state.tile([NLANES, NX], f32)
        u_lm = state.tile([NLANES, NU], f32)
        nu_lm = state.tile([NLANES, NX], f32)
        s_lm = state.tile([NLANES, M2], f32)
        lam_lm = state.tile([NLANES, M2], f32)
        x_cm = state.tile([NX, NLANES], f32)
        u_cm = state.tile([NU, NLANES], f32)
        nu_cm = state.tile([NX, NLANES], f32)
        Lb_cm = state.tile([NX, NLANES], f32)
        Pp_cm = state.tile([65, NLANES], f32)
        P_all = state.tile([NLANES, H * 64], f32)
        p_all = state.tile([NLANES, H * X], f32)
        W_st = state.tile([NLANES, H * 32], f32)
        GB_st = state.tile([NLANES, H * U], f32)
        AUG = state.tile([NLANES, 64], f32)
        SROW = state.tile([NLANES, 16], f32)
        LM = state.tile([NLANES, 320], f32)
        MW_all = state.tile([NLANES, H * 64], f32)
        C_all = state.tile([NLANES, NX], f32)
        dx_allz = state.tile([NLANES, 8 + NX], f32)
        du_all = state.tile([NLANES, NU], f32)
        dnu_lm = state.tile([NLANES, NX], f32)
        ds_lm = state.tile([NLANES, M2], f32)
        dlam_lm = state.tile([NLANES, M2], f32)
        RATZ = state.tile([NLANES, 2 * M2], f32)
        g_cm = state.tile([NX, NLANES], f32)

        for t in (x_lm, u_lm, nu_lm, x_cm, u_cm, nu_cm):
            nc.vector.memset(t[:], 0.0)
        nc.vector.memset(s_lm[:], 1.0)
        nc.vector.memset(lam_lm[:], 1.0)
        nc.vector.memset(Pp_cm[64:65, :], 1.0)
        nc.vector.memset(dx_allz[:, 0:8], 0.0)

        # ---- init: Lb = Linv @ (l_aug x0 + C_aug fc)
        p_Lb = ps.tile([NX, NLANES], f32, tag="p_g")
        nc.tensor.matmul(p_Lb[:], ct["WinLb"][:], ct["xin"][:])
        nc.vector.tensor_copy(Lb_cm[:], p_Lb[:])

        r3 = lambda ap, a, b: ap.rearrange("p (a b) -> p a b", b=b)

        for it in range(n_iters):
            last = it == n_iters - 1
            if True:
                # ================= A: comp-major matmuls =================
                p_g = ps.tile([NX, NLANES], f32)
                nc.tensor.matmul(p_g[:], ct["FT"][:], u_cm[:])
                t_g = wk.tile([NX, NLANES], f32)
                nc.vector.tensor_sub(t_g[:], p_g[:], x_cm[:])
                nc.vector.tensor_add(g_cm[:], t_g[:], Lb_cm[:])

                p_lm = ps.tile([NLANES, 320], f32)
                nc.tensor.matmul(p_lm[:, 0:128], x_cm[:], ct["nQaug"][:],
                                 start=True, stop=False)
                nc.tensor.matmul(p_lm[:, 0:128], nu_cm[:], ct["nL"][:],
                                 start=False, stop=True)
                nc.tensor.matmul(p_lm[:, 128:256], g_cm[:], ct["LTr"][:])
                nc.tensor.matmul(p_lm[:, 256:320], u_cm[:], ct["nR"][:],
                                 start=True, stop=False)
                nc.tensor.matmul(p_lm[:, 256:320], nu_cm[:], ct["Br"][:],
                                 start=False, stop=True)
                nc.vector.tensor_copy(LM[:], p_lm[:])
                r1x = LM[:, 0:128]
                r2v = LM[:, 128:256]
                mrdu = LM[:, 256:320]

                # ================= C: lane-major elementwise =============
                srec = wk.tile([NLANES, M2], f32)
                nc.vector.reciprocal(srec[:], s_lm[:])
                slam = wk.tile([NLANES, M2], f32)
                nc.vector.tensor_mul(slam[:], s_lm[:], lam_lm[:])
                sigmu = wk.tile([NLANES, 1], f32)
                nc.vector.tensor_reduce(sigmu[:], slam[:], AX.X, AL.add)
                nc.vector.tensor_scalar(out=sigmu[:], in0=sigmu[:],
                                        scalar1=SIGMA / M2, scalar2=None,
                                        op0=AL.mult)
                r_i = wk.tile([NLANES, M2], f32)
                nc.vector.scalar_tensor_tensor(
                    out=r_i[:, 0:NU], in0=s_lm[:, 0:NU], scalar=1.0,
                    in1=u_lm[:], op0=AL.subtract, op1=AL.add)
                nc.vector.scalar_tensor_tensor(
                    out=r_i[:, NU:M2], in0=s_lm[:, NU:M2], scalar=1.0,
                    in1=u_lm[:], op0=AL.subtract, op1=AL.subtract)
                r_c = wk.tile([NLANES, M2], f32)
                nc.vector.tensor_scalar(out=r_c[:], in0=slam[:],
                                        scalar1=sigmu[:], scalar2=None,
                                        op0=AL.subtract)
                dv = wk.tile([NLANES, M2], f32)
                nc.vector.tensor_mul(dv[:], lam_lm[:], srec[:])
                wt = wk.tile([NLANES, NU], f32)
                nc.vector.tensor_add(wt[:], dv[:, 0:NU], dv[:, NU:M2])
                wv = wk.tile([NLANES, M2], f32)
                nc.vector.tensor_mul(wv[:], lam_lm[:], r_i[:])
                nc.vector.tensor_sub(wv[:], wv[:], r_c[:])
                nc.vector.tensor_mul(wv[:], wv[:], srec[:])
                wl = wk.tile([NLANES, M2], f32)
                nc.vector.tensor_add(wl[:], wv[:], lam_lm[:])
                r1u = wk.tile([NLANES, NU], f32)
                nc.vector.tensor_sub(r1u[:], mrdu, wl[:, 0:NU])
                nc.vector.tensor_add(r1u[:], r1u[:], wl[:, NU:M2])

                # ================= D: backward Riccati ===================
                nc.vector.tensor_copy(P_all[:, (H - 1) * 64:], ct["vecQ"][:])
                nc.vector.tensor_copy(p_all[:, (H - 1) * X:],
                                      r1x[:, (H - 1) * X:])
                for tau in range(H - 1, -1, -1):
                    p_t = ps.tile([NLANES, 128], f32, bufs=2)
                    if tau == H - 1:
                        nc.vector.tensor_copy(p_t[:, 64:128],
                                              ct["SL0"][:, 64:128])
                        SLvec = ct["SL0"][:, 0:64]
                    else:
                        nc.tensor.matmul(p_t[:], Pp_cm[0:65, :],
                                         ct["Wsand"][:])
                        SLvec = p_t[:, 0:64]
                    AUGp = p_t[:, 64:128]
                    nc.vector.tensor_copy(AUG[:], AUGp)
                    if cfg.get("pchain", "v") == "g":
                        GS = wk.tile([NLANES, 32], f32, tag="GS")
                        nc.vector.tensor_copy(
                            r3(GS[:], 4, 8), r3(AUG[:], 4, 16)[:, :, 0:8])
                    Plm = r3(P_all[:, tau * 64:(tau + 1) * 64], 8, 8)
                    # m = p - P r2_t ; (A^T;B^T) m ; beta   (on GPSIMD)
                    tPK = wk.tile([NLANES, 64], f32, tag="tPK")
                    eng('mchain').tensor_mul(
                        r3(tPK[:], 8, 8), Plm,
                        r2v[:, tau * X:(tau + 1) * X].unsqueeze(1)
                        .broadcast_to([NLANES, 8, 8]))
                    mv = wk.tile([NLANES, X], f32, tag="mv")
                    nc.vector.tensor_reduce(mv[:].unsqueeze(2),
                                            r3(tPK[:], 8, 8), AX.X, AL.add)
                    eng('mchain').tensor_sub(mv[:],
                                         p_all[:, tau * X:(tau + 1) * X],
                                         mv[:])
                    tAB = wk.tile([NLANES, 96], f32, tag="tAB")
                    eng('mchain').tensor_mul(
                        r3(tAB[:], 12, 8), r3(ct["ABrep"][:], 12, 8),
                        mv[:].unsqueeze(1).broadcast_to([NLANES, 12, 8]))
                    ABm = wk.tile([NLANES, 12], f32, tag="ABm")
                    nc.vector.tensor_reduce(ABm[:].unsqueeze(2),
                                            r3(tAB[:], 12, 8), AX.X, AL.add)
                    # AUG (psum, in-place): [beta | G | Lam] per row
                    nc.vector.tensor_add(AUG[:, 12:64:17], AUG[:, 12:64:17],
                                         wt[:, tau * U:(tau + 1) * U])
                    beta_t = wk.tile([NLANES, U], f32, tag="beta_t")
                    nc.vector.tensor_add(beta_t[:], ABm[:, 8:12],
                                         r1u[:, tau * U:(tau + 1) * U])
                    # GJ elimination (4 pivots; pivot of row k at col 9+k)
                    for k in range(4):
                        rk = wk.tile([NLANES, 1], f32, tag="rk")
                        nc.vector.reciprocal(rk[:],
                                             AUG[:, k * 17 + 12:k * 17 + 13])
                        nc.vector.tensor_scalar(
                            out=SROW[:], in0=AUG[:, k * 16:(k + 1) * 16],
                            scalar1=rk[:], scalar2=None, op0=AL.mult)
                        tGJ = wk.tile([NLANES, 64], f32, tag="tGJ")
                        nc.vector.tensor_mul(
                            r3(tGJ[:], 4, 16),
                            r3(AUG[:], 4, 16)[:, :, k + 12:k + 13]
                            .broadcast_to([NLANES, 4, 16]),
                            SROW[:].unsqueeze(1).broadcast_to([NLANES, 4, 16]))
                        nc.vector.tensor_sub(AUG[:], AUG[:], tGJ[:])
                        nc.vector.tensor_copy(AUG[:, k * 16:(k + 1) * 16],
                                              SROW[:])
                    nc.vector.tensor_copy(
                        r3(W_st[:, tau * 32:(tau + 1) * 32], 4, 8),
                        r3(AUG[:], 4, 16)[:, :, 0:8])
                    # Gamma beta (Gamma = AUG[:, :, 8:12])
                    tGB = wk.tile([NLANES, 16], f32, tag="tGB")
                    nc.vector.tensor_mul(
                        r3(tGB[:], 4, 4), r3(AUG[:], 4, 16)[:, :, 8:12],
                        beta_t[:].unsqueeze(1).broadcast_to([NLANES, 4, 4]))
                    nc.vector.tensor_reduce(
                        GB_st[:, tau * U:(tau + 1) * U].unsqueeze(2),
                        r3(tGB[:], 4, 4), AX.X, AL.add)
                    if tau > 0:
                        # closed-loop M_{tau} = A - B W_{tau}  (GPSIMD)
                        tBW = wk.tile([NLANES, 256], f32, tag="tBW")
                        eng('Mchain').tensor_mul(
                            tBW[:].rearrange("p (i j k) -> p i j k", j=8, k=4),
                            r3(ct["Brep2"][:], 8, 4).unsqueeze(2)
                            .broadcast_to([NLANES, 8, 8, 4]),
                            r3(W_st[:, tau * 32:(tau + 1) * 32], 4, 8)
                            .rearrange("p k j -> p j k").unsqueeze(1)
                            .broadcast_to([NLANES, 8, 8, 4]))
                        rBW = wk.tile([NLANES, 64], f32, tag="rBW")
                        nc.vector.tensor_reduce(
                            rBW[:].rearrange("p (i j) -> p i j", j=8)
                            .unsqueeze(3),
                            tBW[:].rearrange("p (i j k) -> p i j k", j=8, k=4),
                            AX.X, AL.add)
                        eng('Mchain').tensor_sub(MW_all[:, tau * 64:(tau + 1) * 64],
                                             ct["Arep2"][:], rBW[:])
                        # P_{tau-1} = (Q + A^T P A) - G^T W
                        if cfg.get("pchain", "v") == "g":
                            GTp = r3(GS[:], 4, 8)
                        else:
                            GTp = r3(AUGp, 4, 16)[:, :, 0:8]
                        GT = GTp.rearrange("p k i -> p i k")
                        tGW = wk.tile([NLANES, 256], f32, tag="tGW")
                        nc.vector.tensor_mul(
                            tGW[:].rearrange("p (i j k) -> p i j k", j=8, k=4),
                            GT.unsqueeze(2).broadcast_to([NLANES, 8, 8, 4]),
                            r3(W_st[:, tau * 32:(tau + 1) * 32], 4, 8)
                            .rearrange("p k j -> p j k").unsqueeze(1)
                            .broadcast_to([NLANES, 8, 8, 4]))
                        PnxtV = r3(P_all[:, (tau - 1) * 64:tau * 64], 8, 8)
                        red = wk.tile([NLANES, 64], f32, tag="redGW")
                        nc.vector.tensor_reduce(
                            red[:].rearrange("p (i j) -> p i j", j=8)
                            .unsqueeze(3),
                            tGW[:].rearrange("p (i j k) -> p i j k", j=8, k=4),
                            AX.X, AL.add)
                        nc.vector.tensor_sub(P_all[:, (tau - 1) * 64:tau * 64],
                                             SLvec, red[:])
                        # p_{tau-1} = r1x_{tau-1} + A^T m - G^T gb
                        tGg = wk.tile([NLANES, 32], f32, tag="tGg")
                        eng('pchain').tensor_mul(
                            r3(tGg[:], 8, 4), GT,
                            GB_st[:, tau * U:(tau + 1) * U].unsqueeze(1)
                            .broadcast_to([NLANES, 8, 4]))
                        Gg = wk.tile([NLANES, X], f32, tag="Gg")
                        nc.vector.tensor_reduce(Gg[:].unsqueeze(2),
                                                r3(tGg[:], 8, 4), AX.X, AL.add)
                        tp = wk.tile([NLANES, X], f32, tag="tp")
                        eng('pchain').tensor_add(tp[:],
                                             r1x[:, (tau - 1) * X:tau * X],
                                             ABm[:, 0:8])
                        eng('pchain').tensor_sub(p_all[:, (tau - 1) * X:tau * X],
                                             tp[:], Gg[:])
                        p_pT = ps.tile([64, NLANES], f32)
                        nc.tensor.transpose(
                            p_pT[:], P_all[:, (tau - 1) * 64:tau * 64],
                            ident[0:NLANES, 0:NLANES])
                        nc.vector.tensor_copy(Pp_cm[0:64, :], p_pT[:])

                # ================= E: forward rollout ====================
                # c_t = B gb_t + r2_t  (batched)
                tBG = wk.tile([NLANES, 512], f32, tag="tBG")
                nc.vector.tensor_mul(
                    tBG[:].rearrange("p (t i k) -> p t i k", i=8, k=4),
                    r3(ct["Brep2"][:], 8, 4).unsqueeze(1)
                    .broadcast_to([NLANES, H, 8, 4]),
                    r3(GB_st[:], H, 4).unsqueeze(2)
                    .broadcast_to([NLANES, H, 8, 4]))
                nc.vector.tensor_reduce(
                    C_all[:].rearrange("p (t i) -> p t i", i=8).unsqueeze(3),
                    tBG[:].rearrange("p (t i k) -> p t i k", i=8, k=4),
                    AX.X, AL.add)
                nc.vector.tensor_add(C_all[:], C_all[:], r2v)
                # dx recursion: dx_{s+1} = M_s dx_s + c_s  (dx_allz[:,0:8]=0)
                for tau in range(H):
                    if tau == 0:
                        nc.vector.tensor_copy(dx_allz[:, 8:16], C_all[:, 0:8])
                        continue
                    tMx = wk.tile([NLANES, 64], f32, tag="tMx")
                    nc.vector.tensor_mul(
                        r3(tMx[:], 8, 8),
                        r3(MW_all[:, tau * 64:(tau + 1) * 64], 8, 8),
                        dx_allz[:, tau * 8:tau * 8 + 8].unsqueeze(1)
                        .broadcast_to([NLANES, 8, 8]))
                    Mx = wk.tile([NLANES, X], f32, tag="Mx")
                    nc.vector.tensor_reduce(Mx[:].unsqueeze(2),
                                            r3(tMx[:], 8, 8), AX.X, AL.add)
                    nc.vector.tensor_add(
                        dx_allz[:, (tau + 1) * 8:(tau + 1) * 8 + 8],
                        Mx[:], C_all[:, tau * X:(tau + 1) * X])
                # du_t = gb_t - W_t dx_{t-1}  (batched)
                tWxb = wk.tile([NLANES, 512], f32, tag="tWxb")
                nc.vector.tensor_mul(
                    tWxb[:].rearrange("p (t k j) -> p t k j", k=4, j=8),
                    W_st[:].rearrange("p (t k j) -> p t k j", k=4, j=8),
                    dx_allz[:, 0:128].rearrange("p (t j) -> p t j", j=8)
                    .unsqueeze(2).broadcast_to([NLANES, H, 4, 8]))
                nc.vector.tensor_reduce(
                    du_all[:].rearrange("p (t k) -> p t k", k=4).unsqueeze(3),
                    tWxb[:].rearrange("p (t k j) -> p t k j", k=4, j=8),
                    AX.X, AL.add)
                nc.vector.tensor_sub(du_all[:], GB_st[:], du_all[:])

                dxv = r3(dx_allz[:, 8:136], H, X)
                duv = r3(du_all[:], H, U)
                # ================= F: dnu = p - P dx  (GPSIMD) ===========
                tPD = wk.tile([NLANES, 1024], f32)
                eng('F').tensor_mul(
                    tPD[:].rearrange("p (t i k) -> p t i k", i=8, k=8),
                    P_all[:].rearrange("p (t i k) -> p t i k", i=8, k=8),
                    dxv.unsqueeze(2).broadcast_to([NLANES, H, 8, 8]))
                nc.vector.tensor_reduce(
                    dnu_lm[:].rearrange("p (t i) -> p t i", i=8).unsqueeze(3),
                    tPD[:].rearrange("p (t i k) -> p t i k", i=8, k=8),
                    AX.X, AL.add)
                eng('F').tensor_sub(dnu_lm[:], p_all[:], dnu_lm[:])

                # ================= G: step, alpha, update ================
                nc.vector.scalar_tensor_tensor(
                    out=r3(ds_lm[:, 0:NU], 16, 4), in0=r3(r_i[:, 0:NU], 16, 4),
                    scalar=-1.0, in1=duv, op0=AL.mult, op1=AL.subtract)
                nc.vector.scalar_tensor_tensor(
                    out=r3(ds_lm[:, NU:M2], 16, 4),
                    in0=r3(r_i[:, NU:M2], 16, 4),
                    scalar=-1.0, in1=duv, op0=AL.mult, op1=AL.add)
                t2 = wk.tile([NLANES, M2], f32)
                nc.vector.tensor_mul(t2[:], lam_lm[:], ds_lm[:])
                nc.vector.tensor_add(t2[:], t2[:], r_c[:])
                nc.vector.scalar_tensor_tensor(
                    out=dlam_lm[:], in0=t2[:], scalar=-1.0, in1=srec[:],
                    op0=AL.mult, op1=AL.mult)
                lrec = wk.tile([NLANES, M2], f32)
                nc.vector.reciprocal(lrec[:], lam_lm[:])
                nc.vector.scalar_tensor_tensor(
                    out=RATZ[:, 0:M2], in0=ds_lm[:], scalar=-1.0, in1=srec[:],
                    op0=AL.mult, op1=AL.mult)
                nc.vector.scalar_tensor_tensor(
                    out=RATZ[:, M2:2 * M2], in0=dlam_lm[:], scalar=-1.0,
                    in1=lrec[:], op0=AL.mult, op1=AL.mult)
                mz = wk.tile([NLANES, 1], f32)
                nc.vector.tensor_reduce(mz[:], RATZ[:], AX.X, AL.max)
                nc.vector.tensor_scalar(out=mz[:], in0=mz[:], scalar1=1e-30,
                                        scalar2=None, op0=AL.max)
                nc.vector.reciprocal(mz[:], mz[:])
                alf = wk.tile([NLANES, 1], f32)
                nc.vector.tensor_scalar(out=alf[:], in0=mz[:], scalar1=0.99,
                                        scalar2=1.0, op0=AL.mult, op1=AL.min)

                for dst, dlt in [(r3(x_lm[:], H, X), dxv),
                                 (r3(u_lm[:], H, U), duv),
                                 (nu_lm[:], dnu_lm[:]),
                                 (s_lm[:], ds_lm[:]),
                                 (lam_lm[:], dlam_lm[:])]:
                    nc.vector.scalar_tensor_tensor(
                        out=dst, in0=dlt, scalar=alf[:], in1=dst,
                        op0=AL.mult, op1=AL.add)

                if not last:
                    for cm, lm, n in [(x_cm, x_lm, NX), (u_cm, u_lm, NU),
                                      (nu_cm, nu_lm, NX)]:
                        p_T = ps.tile([NX, NLANES], f32, tag="p_T")
                        nc.tensor.transpose(p_T[0:n, :], lm[:],
                                            ident[0:NLANES, 0:NLANES])
                        nc.vector.tensor_copy(cm[:], p_T[0:n, :])

        nc.sync.dma_start(x_out[:], x_lm[:])
        nc.sync.dma_start(u_out[:], u_lm[:])
        if dbg:
            for nm, tile in [("LM", LM), ("wt", wt), ("r1u", r1u),
                             ("w", wv), ("r_c", r_c), ("P_all", P_all),
                             ("p_all", p_all), ("W_st", W_st),
                             ("GB_st", GB_st), ("dx", dx_allz),
                             ("du", du_all), ("dnu", dnu_lm),
                             ("ds", ds_lm), ("dlam", dlam_lm),
                             ("alf", alf), ("s", s_lm),
                             ("lam", lam_lm)]:
                od = nc.dram_tensor(f"dbg_{nm}", list(tile.shape), f32,
                                    kind="ExternalOutput")
                nc.sync.dma_start(od[:], tile[:])

    nc.compile()
    return nc


# ---------------------------------------------------------------- entry point
_CACHE = {}


def _make_runner(nc, n_cores=NCORES):
    """Like bass2jax.run_bass_via_pjrt, but the jitted executable is built
    once and reused across calls (the stock helper re-traces and reloads the
    NEFF on every invocation)."""
    import jax
    import concourse.mybir as mybir
    from concourse import bass2jax
    from jax.experimental.shard_map import shard_map
    from jax.sharding import Mesh, PartitionSpec

    bass2jax.install_neuronx_cc_hook()
    partition_name = (nc.partition_id_tensor.name
                      if nc.partition_id_tensor else None)
    in_names, out_names, out_avals, zero_outs = [], [], [], []
    for alloc in nc.m.functions[0].allocations:
        if not isinstance(alloc, mybir.MemoryLocationSet):
            continue
        name = alloc.memorylocations[0].name
        if alloc.kind == "ExternalInput":
            if name != partition_name:
                in_names.append(name)
        elif alloc.kind == "ExternalOutput":
            shape = tuple(alloc.tensor_shape)
            dtype = mybir.dt.np(alloc.dtype)
            out_names.append(name)
            out_avals.append(jax.core.ShapedArray(shape, dtype))
            zero_outs.append(np.zeros(shape, dtype))
    n_params = len(in_names)
    n_outs = len(out_avals)
    all_in_names = list(in_names) + list(out_names)
    if partition_name is not None:
        all_in_names.append(partition_name)
    donate = tuple(range(n_params, n_params + n_outs))

    def _body(*args):
        operands = list(args)
        if partition_name is not None:
            operands.append(bass2jax.partition_id_tensor())
        outs = bass2jax._bass_exec_p.bind(
            *operands, out_avals=tuple(out_avals),
            in_names=tuple(all_in_names), out_names=tuple(out_names),
            lowering_input_output_aliases=(), sim_require_finite=True,
            sim_require_nnan=True, nc=nc)
        return tuple(outs)

    devices = jax.devices()[:n_cores]
    mesh = Mesh(np.asarray(devices), ("core",))
    sharded = jax.jit(
        shard_map(_body, mesh=mesh,
                  in_specs=(PartitionSpec("core"),) * (n_params + n_outs),
                  out_specs=(PartitionSpec("core"),) * n_outs,
                  check_rep=False),
        donate_argnums=donate, keep_unused=True)

    def run(in_maps):
        per_core = [[np.asarray(m[nm]) for nm in in_names] for m in in_maps]
        concat_in = [np.concatenate([per_core[c][i] for c in range(n_cores)],
                                    axis=0) for i in range(n_params)]
        concat_zeros = [np.zeros((n_cores * z.shape[0], *z.shape[1:]), z.dtype)
                        for z in zero_outs]
        out_arrs = sharded(*concat_in, *concat_zeros)
        return [{nm: np.asarray(out_arrs[i]).reshape(n_cores,
                                                     *out_avals[i].shape)[c]
                 for i, nm in enumerate(out_names)}
                for c in range(n_cores)]

    return run


def _pack_inputs(x0, forecast, consts):
    b = x0.shape[0]
    xin = np.concatenate([np.asarray(x0, np.float32).reshape(b, X),
                          np.asarray(forecast, np.float32).reshape(b, H * SD)],
                         axis=1)                      # [b, 40]
    per_core = []
    for c in range(NCORES):
        sl = xin[c * NLANES:(c + 1) * NLANES]          # [16, 40]
        m = {k: v for k, v in consts.items()}
        m["xin"] = np.ascontiguousarray(sl.T)          # [40, 16]
        per_core.append(m)
    return per_core


def kernel(x0, forecast, A, B, C, Q, R):
    if "run" not in _CACHE:
        _CACHE["prog"] = build_program()
        _CACHE["run"] = _make_runner(_CACHE["prog"])
    consts = build_host_consts(A, B, C, Q, R)
    in_maps = _pack_inputs(x0, forecast, consts)
    results = _CACHE["run"](in_maps)
    xs, us = [], []
    for c in range(NCORES):
        xs.append(results[c]["x_out"].reshape(NLANES, H, X))
        us.append(results[c]["u_out"].reshape(NLANES, H, U))
    x_sol = np.concatenate(xs, axis=0).astype(np.float32)
    u_sol = np.concatenate(us, axis=0).astype(np.float32)
    return (x_sol, u_sol)
